# revision 20
# baseline (speedup 1.0000x reference)
"""Dice metric kernel for Trainium2 (Bass/Tile), 8-core data parallel.

Reference computation (per sample b):
    pred = argmax_c logits[b, :, h, w]   (softmax is monotonic -> argmax of logits)
    For classes c = 1..7:
        tps_c  = #{pred == c  and  tgt == c}
        pmc_c  = #{pred == c}
        tmc_c  = #{tgt == c}
        dice_c = 2*tps_c / (pmc_c + tmc_c + 1e-5)
    out[b] = mean_c dice_c

Device mapping (per core: 2 samples, fp16 planes [128, 2048]):
  - DVE: batched max tree (3 ops), batched is_ge -> pm masks (1 op),
    7 fused (t==c)*pm ops with free-dim accumulation (tps).
  - ACT: target histogram via Relu moments: R_k = sum relu(t-k), k=0..6;
    tgt_count_c = R_{c-1} - 2 R_c + R_{c+1}  (exact for integer t).
  - PE:  pred counts via ones-matmul over pm chunks; cross-partition sums.

Sharding: batch 16 -> 2 samples per core on 8 cores; host concatenates.
"""

import numpy as np

import concourse.bacc as bacc
import concourse.mybir as mybir
import concourse.tile as tile
from concourse.bass_utils import run_bass_kernel_spmd

B, C, H, W = 16, 8, 512, 512
NCORES = 8
BPC = B // NCORES          # samples per core
P = 128                    # SBUF partitions
F = (H * W) // P           # free dim per plane (2048)
EPS = 1e-5

_f32 = mybir.dt.float32
_f16 = mybir.dt.float16
_alu = mybir.AluOpType
_act = mybir.ActivationFunctionType


def _build_nc():
    nc = bacc.Bacc(None, target_bir_lowering=False, debug=False)
    x_dram = nc.dram_tensor("x", [BPC, C, P, F], _f16, kind="ExternalInput")
    t_dram = nc.dram_tensor("t", [BPC, P, F], _f16, kind="ExternalInput")
    o_dram = nc.dram_tensor("o", [1, BPC], _f32, kind="ExternalOutput")
    i7_dram = nc.dram_tensor("i7", [8, 8], _f32, kind="ExternalInput")

    with tile.TileContext(nc) as tc:
        with (
            tc.tile_pool(name="xp", bufs=2) as xp,
            tc.tile_pool(name="mt", bufs=2) as mtp,
            tc.tile_pool(name="wk", bufs=2) as wk,
            tc.tile_pool(name="ac", bufs=2) as acp,
            tc.tile_pool(name="cst", bufs=1) as cst,
            tc.tile_pool(name="ps", bufs=2, space="PSUM") as ps,
        ):
            ones16 = cst.tile([P, 1], _f16)
            nc.gpsimd.memset(ones16[:], 1.0)
            ones32 = cst.tile([P, 1], _f32)
            nc.gpsimd.memset(ones32[:], 1.0)
            i7 = cst.tile([8, 8], _f32)
            nc.sync.dma_start(i7[:], i7_dram[:])
            kbias = cst.tile([P, 8], _f32)
            for k in range(7):
                nc.gpsimd.memset(kbias[:, k : k + 1], -float(k))
            ecs = cst.tile([P, 7, 8], _f16)
            nc.gpsimd.memset(ecs[:], 0.0)
            for ci in range(7):
                nc.gpsimd.memset(ecs[:, ci, ci : ci + 1], 1.0)
            osb = cst.tile([1, BPC], _f32)

            NHMAX = 2
            for b in range(BPC):
                BOUNDS = [0, 512, F] if b == 0 else [0, F]
                NH = len(BOUNDS) - 1
                xbig = xp.tile([P, C, F], _f16, tag="x")
                tt = xp.tile([P, F], _f16, tag="t")
                # acc cols (per chunk h at offset 16*h):
                #   0..6 = R_0..R_6 (ACT Relu sums), 7,8 = zero, 9..15 = tps_1..7
                acc = acp.tile([P, 16 * NHMAX], _f32, tag="acc")
                nc.gpsimd.memset(acc[:], 0.0)
                pp = ps.tile([8, 512], _f32, tag="pp")

                xr = x_dram[b].rearrange("c p f -> p c f")
                for h in range(NH):
                    hs = slice(BOUNDS[h], BOUNDS[h + 1])
                    Fh = BOUNDS[h + 1] - BOUNDS[h]
                    nc.sync.dma_start(xbig[:, :, hs], xr[:, :, hs])
                    nc.sync.dma_start(tt[:, hs], t_dram[b, :, hs])

                    # max over the 8 class planes: 3 batched ops
                    l1 = mtp.tile([P, 4, Fh], _f16, tag="l1")
                    nc.vector.tensor_max(l1[:], xbig[:, 0:4, hs], xbig[:, 4:8, hs])
                    l2 = mtp.tile([P, 2, Fh], _f16, tag="l2")
                    nc.vector.tensor_max(l2[:], l1[:, 0:2, :], l1[:, 2:4, :])
                    mx = wk.tile([P, Fh], _f16, tag="mx")
                    nc.vector.tensor_max(mx[:], l2[:, 0, :], l2[:, 1, :])

                    # pm masks for classes 1..7 in one batched op
                    pm = mtp.tile([P, 7, Fh], _f16, tag="pm")
                    mxb = mx.rearrange("p (o f) -> p o f", o=1).broadcast_to(
                        (P, 7, Fh)
                    )
                    nc.vector.tensor_tensor(
                        out=pm[:], in0=xbig[:, 1:8, hs], in1=mxb, op=_alu.is_ge
                    )

                    for k in range(7):
                        aj = wk.tile([P, Fh], _f16, tag="aj")
                        nc.scalar.activation(
                            aj[:],
                            tt[:, hs],
                            _act.Relu,
                            bias=kbias[:, k : k + 1],
                            scale=1.0,
                            accum_out=acc[:, 16 * h + k : 16 * h + k + 1],
                        )

                    for ci in range(7):
                        junk = wk.tile([P, Fh], _f16, tag="junk")
                        nc.vector.scalar_tensor_tensor(
                            out=junk[:],
                            in0=tt[:, hs],
                            scalar=float(ci + 1),
                            in1=pm[:, ci, :],
                            op0=_alu.is_equal,
                            op1=_alu.mult,
                            accum_out=acc[:, 16 * h + 9 + ci : 16 * h + 10 + ci],
                        )

                    # pred counts: pp[ci, :] += ecs[ci]^T @ pm[:, ci, chunk]
                    for ci in range(7):
                        for j in range(Fh // 512):
                            nc.tensor.matmul(
                                pp[:, :],
                                ecs[:, ci, :],
                                pm[:, ci, j * 512 : (j + 1) * 512],
                                start=(h == 0 and ci == 0 and j == 0),
                                stop=(
                                    h == NH - 1
                                    and ci == 6
                                    and j == Fh // 512 - 1
                                ),
                            )

                pr = acp.tile([8, 1], _f32, tag="pr")
                aj2 = wk.tile([8, 512], _f32, tag="aj2")
                nc.scalar.activation(
                    aj2[:], pp[:, :], _act.Copy, accum_out=pr[:, :]
                )

                # cross-partition sums
                pt1 = ps.tile([1, 16 * NHMAX], _f32, tag="pt1")
                nc.tensor.matmul(pt1[:], ones32[:], acc[:, :], start=True, stop=True)
                pt2 = ps.tile([1, 8], _f32, tag="pt2")
                nc.tensor.matmul(
                    pt2[0:1, 0:7], pr[0:7, :], i7[0:7, 0:7], start=True, stop=True
                )

                # epilogue on partition 0
                cnt0 = wk.tile([1, 16 * NHMAX], _f32, tag="cnt0")
                nc.scalar.copy(cnt0[:], pt1[:])
                cnt1 = wk.tile([1, 16], _f32, tag="cnt1")
                nc.vector.tensor_add(
                    cnt1[:], cnt0[0:1, 0:16], cnt0[0:1, 16:32]
                )
                cnt2 = wk.tile([1, 8], _f32, tag="cnt2")
                nc.scalar.copy(cnt2[0:1, 0:7], pt2[0:1, 0:7])
                v = wk.tile([1, 8], _f32, tag="v")
                nc.vector.tensor_sub(v[:], cnt1[0:1, 0:8], cnt1[0:1, 1:9])
                tmv = wk.tile([1, 8], _f32, tag="tmv")
                nc.vector.tensor_sub(tmv[0:1, 0:7], v[0:1, 0:7], v[0:1, 1:8])
                den = wk.tile([1, 8], _f32, tag="den")
                nc.vector.scalar_tensor_tensor(
                    out=den[0:1, 0:7],
                    in0=cnt2[0:1, 0:7],
                    scalar=EPS,
                    in1=tmv[0:1, 0:7],
                    op0=_alu.add,
                    op1=_alu.add,
                )
                rec = wk.tile([1, 8], _f32, tag="rec")
                nc.vector.reciprocal(rec[0:1, 0:7], den[0:1, 0:7])
                dice = wk.tile([1, 8], _f32, tag="dice")
                nc.vector.scalar_tensor_tensor(
                    out=dice[0:1, 0:7],
                    in0=cnt1[0:1, 9:16],
                    scalar=2.0 / 7.0,
                    in1=rec[0:1, 0:7],
                    op0=_alu.mult,
                    op1=_alu.mult,
                    accum_out=osb[0:1, b : b + 1],
                )

            nc.sync.dma_start(o_dram[:], osb[:])

    nc.compile()
    return nc


_NC_CACHE = {}


def _get_nc():
    if "nc" not in _NC_CACHE:
        _NC_CACHE["nc"] = _build_nc()
    return _NC_CACHE["nc"]


def make_in_maps(inputs: np.ndarray, targets: np.ndarray) -> list:
    x = (
        np.ascontiguousarray(inputs, dtype=np.float32)
        .astype(np.float16)
        .reshape(NCORES, BPC, C, P, F)
    )
    t = (
        np.ascontiguousarray(targets)
        .astype(np.float16)
        .reshape(NCORES, BPC, P, F)
    )
    eye = np.eye(8, dtype=np.float32)
    return [{"x": x[i], "t": t[i], "i7": eye} for i in range(NCORES)]


def kernel(inputs: np.ndarray, targets: np.ndarray) -> np.ndarray:
    in_maps = make_in_maps(inputs, targets)
    nc = _get_nc()
    res = run_bass_kernel_spmd(nc, in_maps, list(range(NCORES)))
    outs = [res.results[i]["o"].reshape(BPC) for i in range(NCORES)]
    return np.concatenate(outs).astype(np.float32)


# revision 21
# speedup vs baseline: 1.0236x; 1.0236x over previous
"""Dice metric kernel for Trainium2 (Bass/Tile), 8-core data parallel.

Reference computation (per sample b):
    pred = argmax_c logits[b, :, h, w]   (softmax is monotonic -> argmax of logits)
    For classes c = 1..7:
        tps_c  = #{pred == c  and  tgt == c}
        pmc_c  = #{pred == c}
        tmc_c  = #{tgt == c}
        dice_c = 2*tps_c / (pmc_c + tmc_c + 1e-5)
    out[b] = mean_c dice_c

Device mapping (per core: 2 samples, fp16 planes [128, 2048]):
  - DVE: batched max tree (3 ops), batched is_ge -> pm masks (1 op),
    7 fused (t==c)*pm ops with free-dim accumulation (tps).
  - ACT: target histogram via Relu moments: R_k = sum relu(t-k), k=0..6;
    tgt_count_c = R_{c-1} - 2 R_c + R_{c+1}  (exact for integer t).
  - PE:  pred counts via ones-matmul over pm chunks; cross-partition sums.

Sharding: batch 16 -> 2 samples per core on 8 cores; host concatenates.
"""

import numpy as np

import concourse.bacc as bacc
import concourse.mybir as mybir
import concourse.tile as tile
from concourse.bass_utils import run_bass_kernel_spmd

B, C, H, W = 16, 8, 512, 512
NCORES = 8
BPC = B // NCORES          # samples per core
P = 128                    # SBUF partitions
F = (H * W) // P           # free dim per plane (2048)
EPS = 1e-5

_f32 = mybir.dt.float32
_f16 = mybir.dt.float16
_alu = mybir.AluOpType
_act = mybir.ActivationFunctionType


def _build_nc():
    nc = bacc.Bacc(None, target_bir_lowering=False, debug=False)
    x_dram = nc.dram_tensor("x", [BPC, C, P, F], _f16, kind="ExternalInput")
    t_dram = nc.dram_tensor("t", [BPC, P, F], _f16, kind="ExternalInput")
    o_dram = nc.dram_tensor("o", [1, BPC], _f32, kind="ExternalOutput")
    i7_dram = nc.dram_tensor("i7", [8, 8], _f32, kind="ExternalInput")

    with tile.TileContext(nc) as tc:
        with (
            tc.tile_pool(name="xp", bufs=2) as xp,
            tc.tile_pool(name="mt", bufs=1) as mtp,
            tc.tile_pool(name="wk", bufs=2) as wk,
            tc.tile_pool(name="ac", bufs=2) as acp,
            tc.tile_pool(name="cst", bufs=1) as cst,
            tc.tile_pool(name="ps", bufs=2, space="PSUM") as ps,
        ):
            ones16 = cst.tile([P, 1], _f16)
            nc.gpsimd.memset(ones16[:], 1.0)
            ones32 = cst.tile([P, 1], _f32)
            nc.gpsimd.memset(ones32[:], 1.0)
            i7 = cst.tile([8, 8], _f32)
            nc.sync.dma_start(i7[:], i7_dram[:])
            kbias = cst.tile([P, 8], _f32)
            for k in range(7):
                nc.gpsimd.memset(kbias[:, k : k + 1], -float(k))
            ecs = cst.tile([P, 7, 8], _f16)
            nc.gpsimd.memset(ecs[:], 0.0)
            for ci in range(7):
                nc.gpsimd.memset(ecs[:, ci, ci : ci + 1], 1.0)
            osb = cst.tile([1, BPC], _f32)

            NHMAX = 2
            for b in range(BPC):
                BOUNDS = [0, 512, F] if b == 0 else [0, F]
                NH = len(BOUNDS) - 1
                xbig = xp.tile([P, C, F], _f16, tag="x")
                tt = xp.tile([P, F], _f16, tag="t")
                # acc cols (per chunk h at offset 16*h):
                #   0..6 = R_0..R_6 (ACT Relu sums), 7,8 = zero, 9..15 = tps_1..7
                acc = acp.tile([P, 16 * NHMAX], _f32, tag="acc")
                nc.gpsimd.memset(acc[:], 0.0)
                pp = ps.tile([8, 512], _f32, tag="pp")

                xr = x_dram[b].rearrange("c p f -> p c f")
                for h in range(NH):
                    hs = slice(BOUNDS[h], BOUNDS[h + 1])
                    Fh = BOUNDS[h + 1] - BOUNDS[h]
                    nc.sync.dma_start(xbig[:, :, hs], xr[:, :, hs])
                    nc.sync.dma_start(tt[:, hs], t_dram[b, :, hs])

                    # max over the 8 class planes: 3 batched ops
                    l1 = mtp.tile([P, 4, Fh], _f16, tag="l1")
                    nc.vector.tensor_max(l1[:], xbig[:, 0:4, hs], xbig[:, 4:8, hs])
                    l2 = mtp.tile([P, 2, Fh], _f16, tag="l2")
                    nc.vector.tensor_max(l2[:], l1[:, 0:2, :], l1[:, 2:4, :])
                    mx = wk.tile([P, Fh], _f16, tag="mx")
                    nc.vector.tensor_max(mx[:], l2[:, 0, :], l2[:, 1, :])

                    # pm masks for classes 1..7 in one batched op
                    pm = mtp.tile([P, 7, Fh], _f16, tag="pm")
                    mxb = mx.rearrange("p (o f) -> p o f", o=1).broadcast_to(
                        (P, 7, Fh)
                    )
                    nc.vector.tensor_tensor(
                        out=pm[:], in0=xbig[:, 1:8, hs], in1=mxb, op=_alu.is_ge
                    )

                    for k in range(7):
                        aj = wk.tile([P, Fh], _f16, tag="aj")
                        nc.scalar.activation(
                            aj[:],
                            tt[:, hs],
                            _act.Relu,
                            bias=kbias[:, k : k + 1],
                            scale=1.0,
                            accum_out=acc[:, 16 * h + k : 16 * h + k + 1],
                        )

                    for ci in range(7):
                        junk = wk.tile([P, Fh], _f16, tag="junk")
                        nc.vector.scalar_tensor_tensor(
                            out=junk[:],
                            in0=tt[:, hs],
                            scalar=float(ci + 1),
                            in1=pm[:, ci, :],
                            op0=_alu.is_equal,
                            op1=_alu.mult,
                            accum_out=acc[:, 16 * h + 9 + ci : 16 * h + 10 + ci],
                        )

                    # pred counts: pp[ci, :] += ecs[ci]^T @ pm[:, ci, chunk]
                    for ci in range(7):
                        for j in range(Fh // 512):
                            nc.tensor.matmul(
                                pp[:, :],
                                ecs[:, ci, :],
                                pm[:, ci, j * 512 : (j + 1) * 512],
                                start=(h == 0 and ci == 0 and j == 0),
                                stop=(
                                    h == NH - 1
                                    and ci == 6
                                    and j == Fh // 512 - 1
                                ),
                            )

                pr = acp.tile([8, 1], _f32, tag="pr")
                aj2 = wk.tile([8, 512], _f32, tag="aj2")
                nc.scalar.activation(
                    aj2[:], pp[:, :], _act.Copy, accum_out=pr[:, :]
                )

                # cross-partition sums
                pt1 = ps.tile([1, 16 * NHMAX], _f32, tag="pt1")
                nc.tensor.matmul(pt1[:], ones32[:], acc[:, :], start=True, stop=True)
                pt2 = ps.tile([1, 8], _f32, tag="pt2")
                nc.tensor.matmul(
                    pt2[0:1, 0:7], pr[0:7, :], i7[0:7, 0:7], start=True, stop=True
                )

                # epilogue on partition 0
                cnt0 = wk.tile([1, 16 * NHMAX], _f32, tag="cnt0")
                nc.scalar.copy(cnt0[:], pt1[:])
                cnt1 = wk.tile([1, 16], _f32, tag="cnt1")
                nc.vector.tensor_add(
                    cnt1[:], cnt0[0:1, 0:16], cnt0[0:1, 16:32]
                )
                cnt2 = wk.tile([1, 8], _f32, tag="cnt2")
                nc.scalar.copy(cnt2[0:1, 0:7], pt2[0:1, 0:7])
                v = wk.tile([1, 8], _f32, tag="v")
                nc.vector.tensor_sub(v[:], cnt1[0:1, 0:8], cnt1[0:1, 1:9])
                tmv = wk.tile([1, 8], _f32, tag="tmv")
                nc.vector.tensor_sub(tmv[0:1, 0:7], v[0:1, 0:7], v[0:1, 1:8])
                den = wk.tile([1, 8], _f32, tag="den")
                nc.vector.scalar_tensor_tensor(
                    out=den[0:1, 0:7],
                    in0=cnt2[0:1, 0:7],
                    scalar=EPS,
                    in1=tmv[0:1, 0:7],
                    op0=_alu.add,
                    op1=_alu.add,
                )
                rec = wk.tile([1, 8], _f32, tag="rec")
                nc.vector.reciprocal(rec[0:1, 0:7], den[0:1, 0:7])
                dice = wk.tile([1, 8], _f32, tag="dice")
                nc.vector.scalar_tensor_tensor(
                    out=dice[0:1, 0:7],
                    in0=cnt1[0:1, 9:16],
                    scalar=2.0 / 7.0,
                    in1=rec[0:1, 0:7],
                    op0=_alu.mult,
                    op1=_alu.mult,
                    accum_out=osb[0:1, b : b + 1],
                )

            nc.sync.dma_start(o_dram[:], osb[:])

    nc.compile()
    return nc


_NC_CACHE = {}


def _get_nc():
    if "nc" not in _NC_CACHE:
        _NC_CACHE["nc"] = _build_nc()
    return _NC_CACHE["nc"]


def make_in_maps(inputs: np.ndarray, targets: np.ndarray) -> list:
    x = (
        np.ascontiguousarray(inputs, dtype=np.float32)
        .astype(np.float16)
        .reshape(NCORES, BPC, C, P, F)
    )
    t = (
        np.ascontiguousarray(targets)
        .astype(np.float16)
        .reshape(NCORES, BPC, P, F)
    )
    eye = np.eye(8, dtype=np.float32)
    return [{"x": x[i], "t": t[i], "i7": eye} for i in range(NCORES)]


def kernel(inputs: np.ndarray, targets: np.ndarray) -> np.ndarray:
    in_maps = make_in_maps(inputs, targets)
    nc = _get_nc()
    res = run_bass_kernel_spmd(nc, in_maps, list(range(NCORES)))
    outs = [res.results[i]["o"].reshape(BPC) for i in range(NCORES)]
    return np.concatenate(outs).astype(np.float32)


# revision 24
# speedup vs baseline: 1.0430x; 1.0190x over previous
"""Dice metric kernel for Trainium2 (Bass/Tile), 8-core data parallel.

Reference computation (per sample b):
    pred = argmax_c logits[b, :, h, w]   (softmax is monotonic -> argmax of logits)
    For classes c = 1..7:
        tps_c  = #{pred == c  and  tgt == c}
        pmc_c  = #{pred == c}
        tmc_c  = #{tgt == c}
        dice_c = 2*tps_c / (pmc_c + tmc_c + 1e-5)
    out[b] = mean_c dice_c

Device mapping (per core: 2 samples, fp16 planes [128, 2048]):
  - DVE: batched max tree (3 ops), batched is_ge -> pm masks (1 op),
    7 fused (t==c)*pm ops with free-dim accumulation (tps).
  - ACT: target histogram via Relu moments: R_k = sum relu(t-k), k=0..6;
    tgt_count_c = R_{c-1} - 2 R_c + R_{c+1}  (exact for integer t).
  - PE:  pred counts via ones-matmul over pm chunks; cross-partition sums.

Sharding: batch 16 -> 2 samples per core on 8 cores; host concatenates.
"""

import numpy as np

import concourse.bacc as bacc
import concourse.mybir as mybir
import concourse.tile as tile
from concourse.bass_utils import run_bass_kernel_spmd

B, C, H, W = 16, 8, 512, 512
NCORES = 8
BPC = B // NCORES          # samples per core
P = 128                    # SBUF partitions
F = (H * W) // P           # free dim per plane (2048)
EPS = 1e-5

_f32 = mybir.dt.float32
_f16 = mybir.dt.float16
_alu = mybir.AluOpType
_act = mybir.ActivationFunctionType


def _build_nc():
    nc = bacc.Bacc(None, target_bir_lowering=False, debug=False)
    x_dram = nc.dram_tensor("x", [BPC, C, P, F], _f16, kind="ExternalInput")
    t_dram = nc.dram_tensor("t", [BPC, P, F], _f16, kind="ExternalInput")
    o_dram = nc.dram_tensor("o", [1, BPC], _f32, kind="ExternalOutput")
    i7_dram = nc.dram_tensor("i7", [8, 8], _f32, kind="ExternalInput")

    with tile.TileContext(nc) as tc:
        with (
            tc.tile_pool(name="xp", bufs=2) as xp,
            tc.tile_pool(name="mt", bufs=1) as mtp,
            tc.tile_pool(name="wk", bufs=2) as wk,
            tc.tile_pool(name="ac", bufs=2) as acp,
            tc.tile_pool(name="cst", bufs=1) as cst,
            tc.tile_pool(name="ps", bufs=2, space="PSUM") as ps,
        ):
            ones16 = cst.tile([P, 1], _f16)
            nc.gpsimd.memset(ones16[:], 1.0)
            ones32 = cst.tile([P, 1], _f32)
            nc.gpsimd.memset(ones32[:], 1.0)
            i7 = cst.tile([8, 8], _f32)
            nc.sync.dma_start(i7[:], i7_dram[:])
            kbias = cst.tile([P, 8], _f32)
            for k in range(7):
                nc.gpsimd.memset(kbias[:, k : k + 1], -float(k))
            ecs = cst.tile([P, 7, 8], _f16)
            nc.gpsimd.memset(ecs[:], 0.0)
            for ci in range(7):
                nc.gpsimd.memset(ecs[:, ci, ci : ci + 1], 1.0)
            osb = cst.tile([1, BPC], _f32)

            NHMAX = 2
            for b in range(BPC):
                BOUNDS = [0, 512, F] if b == 0 else [0, F]
                NH = len(BOUNDS) - 1
                xbig = xp.tile([P, C, F], _f16, tag="x")
                tt = xp.tile([P, F], _f16, tag="t")
                # acc cols (per chunk h at offset 16*h):
                #   0..6 = R_0..R_6 (ACT Relu sums), 7,8 = zero, 9..15 = tps_1..7
                acc = acp.tile([P, 16 * NHMAX], _f32, tag="acc")
                nc.gpsimd.memset(acc[:], 0.0)
                pp = ps.tile([8, 512], _f32, tag="pp")

                xr = x_dram[b].rearrange("c p f -> p c f")
                for h in range(NH):
                    hs = slice(BOUNDS[h], BOUNDS[h + 1])
                    Fh = BOUNDS[h + 1] - BOUNDS[h]
                    nc.sync.dma_start(xbig[:, :, hs], xr[:, :, hs])
                    nc.sync.dma_start(tt[:, hs], t_dram[b, :, hs])

                    # max over the 8 class planes: 3 batched ops
                    l1 = mtp.tile([P, 4, Fh], _f16, tag="l1")
                    nc.vector.tensor_max(l1[:], xbig[:, 0:4, hs], xbig[:, 4:8, hs])
                    l2 = mtp.tile([P, 2, Fh], _f16, tag="l2")
                    nc.vector.tensor_max(l2[:], l1[:, 0:2, :], l1[:, 2:4, :])
                    mx = wk.tile([P, Fh], _f16, tag="mx")
                    nc.vector.tensor_max(mx[:], l2[:, 0, :], l2[:, 1, :])

                    # pm masks for classes 1..7 in one batched op
                    pm = mtp.tile([P, 7, Fh], _f16, tag="pm")
                    mxb = mx.rearrange("p (o f) -> p o f", o=1).broadcast_to(
                        (P, 7, Fh)
                    )
                    nc.vector.tensor_tensor(
                        out=pm[:], in0=xbig[:, 1:8, hs], in1=mxb, op=_alu.is_ge
                    )

                    for k in range(7):
                        aj = wk.tile([P, Fh], _f16, tag="aj")
                        nc.scalar.activation(
                            aj[:],
                            tt[:, hs],
                            _act.Relu,
                            bias=kbias[:, k : k + 1],
                            scale=1.0,
                            accum_out=acc[:, 16 * h + k : 16 * h + k + 1],
                        )

                    for ci in range(7):
                        junk = wk.tile([P, Fh], _f16, tag="junk")
                        nc.vector.scalar_tensor_tensor(
                            out=junk[:],
                            in0=tt[:, hs],
                            scalar=float(ci + 1),
                            in1=pm[:, ci, :],
                            op0=_alu.is_equal,
                            op1=_alu.mult,
                            accum_out=acc[:, 16 * h + 9 + ci : 16 * h + 10 + ci],
                        )

                    # pred counts: pp[ci, :] += ecs[ci]^T @ pm[:, ci, chunk]
                    for ci in range(7):
                        starts = list(range(0, Fh, 512))
                        for j, js in enumerate(starts):
                            n = min(512, Fh - js)
                            nc.tensor.matmul(
                                pp[:, 0:n],
                                ecs[:, ci, :],
                                pm[:, ci, js : js + n],
                                start=(h == 0 and ci == 0 and j == 0),
                                stop=(
                                    h == NH - 1
                                    and ci == 6
                                    and j == len(starts) - 1
                                ),
                            )

                pr = acp.tile([8, 1], _f32, tag="pr")
                aj2 = wk.tile([8, 512], _f32, tag="aj2")
                nc.scalar.activation(
                    aj2[:], pp[:, :], _act.Copy, accum_out=pr[:, :]
                )

                # cross-partition sums
                pt1 = ps.tile([1, 16 * NHMAX], _f32, tag="pt1")
                nc.tensor.matmul(pt1[:], ones32[:], acc[:, :], start=True, stop=True)
                pt2 = ps.tile([1, 8], _f32, tag="pt2")
                nc.tensor.matmul(
                    pt2[0:1, 0:7], pr[0:7, :], i7[0:7, 0:7], start=True, stop=True
                )

                # epilogue on partition 0
                cnt0 = wk.tile([1, 16 * NHMAX], _f32, tag="cnt0")
                nc.scalar.copy(cnt0[:], pt1[:])
                cnt1 = wk.tile([1, 16], _f32, tag="cnt1")
                nc.vector.tensor_add(
                    cnt1[:], cnt0[0:1, 0:16], cnt0[0:1, 16:32]
                )
                cnt2 = wk.tile([1, 8], _f32, tag="cnt2")
                nc.scalar.copy(cnt2[0:1, 0:7], pt2[0:1, 0:7])
                v = wk.tile([1, 8], _f32, tag="v")
                nc.vector.tensor_sub(v[:], cnt1[0:1, 0:8], cnt1[0:1, 1:9])
                tmv = wk.tile([1, 8], _f32, tag="tmv")
                nc.vector.tensor_sub(tmv[0:1, 0:7], v[0:1, 0:7], v[0:1, 1:8])
                den = wk.tile([1, 8], _f32, tag="den")
                nc.vector.scalar_tensor_tensor(
                    out=den[0:1, 0:7],
                    in0=cnt2[0:1, 0:7],
                    scalar=EPS,
                    in1=tmv[0:1, 0:7],
                    op0=_alu.add,
                    op1=_alu.add,
                )
                rec = wk.tile([1, 8], _f32, tag="rec")
                nc.vector.reciprocal(rec[0:1, 0:7], den[0:1, 0:7])
                dice = wk.tile([1, 8], _f32, tag="dice")
                nc.vector.scalar_tensor_tensor(
                    out=dice[0:1, 0:7],
                    in0=cnt1[0:1, 9:16],
                    scalar=2.0 / 7.0,
                    in1=rec[0:1, 0:7],
                    op0=_alu.mult,
                    op1=_alu.mult,
                    accum_out=osb[0:1, b : b + 1],
                )

            nc.sync.dma_start(o_dram[:], osb[:])

    nc.compile()
    return nc


_NC_CACHE = {}


def _get_nc():
    if "nc" not in _NC_CACHE:
        _NC_CACHE["nc"] = _build_nc()
    return _NC_CACHE["nc"]


def make_in_maps(inputs: np.ndarray, targets: np.ndarray) -> list:
    x = (
        np.ascontiguousarray(inputs, dtype=np.float32)
        .astype(np.float16)
        .reshape(NCORES, BPC, C, P, F)
    )
    t = (
        np.ascontiguousarray(targets)
        .astype(np.float16)
        .reshape(NCORES, BPC, P, F)
    )
    eye = np.eye(8, dtype=np.float32)
    return [{"x": x[i], "t": t[i], "i7": eye} for i in range(NCORES)]


def kernel(inputs: np.ndarray, targets: np.ndarray) -> np.ndarray:
    in_maps = make_in_maps(inputs, targets)
    nc = _get_nc()
    res = run_bass_kernel_spmd(nc, in_maps, list(range(NCORES)))
    outs = [res.results[i]["o"].reshape(BPC) for i in range(NCORES)]
    return np.concatenate(outs).astype(np.float32)
